# revision 12
# baseline (speedup 1.0000x reference)
"""DeformableConv2D Trainium2 Bass kernel (v3).

Problem: x[4,64,64,256] f32, w_offset[3,3,256,27], b_offset[27], filt[256,256,3,3]
  -> out[4,64,64,256] f32  (3x3 deformable conv, DG=1, SAME padding)

Sharding: 8 cores = (batch b = core//2) x (image-row half = core%2).
Each core computes 32 output rows (2048 pixels) of its batch element.

Key layout trick: the host stages a zero-padded, *paired-row* bf16 copy of
the image: record r=(y,x) holds channels of pixels (y,x) AND (y+1,x).  One
2KB dma_gather descriptor per (tap, pixel) then fetches all 4 bilinear
corners [v00|v10|v01|v11] at once, pixel-major.

v3 structure (front-end split so gathers start ~30us in):
  split s=0 covers pixel group g=0, split s=1 covers g=1..3.
  P3  offset conv per row band (PE, bf16): wi_cm [27, 2048]
  PT  PE-transpose -> pixel-major wi_pm
  P4  per-split DVE: clamp, floor, corner weights, gather indices
  IDX per-split: partition-fold DMAs (288B runs) + DVE shuffle +
      parallel replication ladder -> idxw [128, 1152]
      split-0 DMAs on gpsimd+sync (early), split-1 on sync only so they
      never block gathers or sampled copies.
  DG  per (g,kk): diag tile [128, 16x128] = identity * w (stride-0 TT),
      all on DVE, pre-built ahead of consumption
  P5  SWDGE dma_gather on 4 queues
  P6  PE matmuls lhsT=gt, rhs=diag(w) accumulating 4 corners in PSUM
      (fused scale+transpose+corner-sum) -> sampled [c, px]
  P7  incremental stage-2: po[tl] += sampled.T @ W right after each
      (kk,ch) sampled tile lands (no end-of-group tail); PSUM->SBUF
      copies all on the scalar engine.
"""

import os
import sys
import numpy as np
import ml_dtypes

sys.path.insert(0, "/opt/trn_rl_repo")

BF16 = ml_dtypes.bfloat16

B, H, W, C, F, K, KK = 4, 64, 64, 256, 256, 3, 9
PAD = 6
Wp = 76
SLAB_ROWS = 45           # 44 addressable + 1 zero guard row
SLAB_PX = SLAB_ROWS * Wp  # 3420
NREC = 44 * Wp            # 3344 addressable paired-row records
NPX = 2048
CLAMP = 4.99

_CACHE = {}
LAST_RESULT = None
DEBUG = bool(int(os.environ.get("KERNEL_DEBUG", "0")))


def _build_nc():
    import concourse.bass as bass
    from concourse import bacc, mybir
    import concourse.tile as tile

    dt = mybir.dt
    Alu = mybir.AluOpType
    Act = mybir.ActivationFunctionType

    nc = bacc.Bacc("TRN2", target_bir_lowering=False, num_swdge_queues=4)

    xrp_d = nc.dram_tensor("xrp", [NREC + 1, 512], dt.bfloat16, kind="ExternalInput")
    xcm_d = nc.dram_tensor("xslab_cm", [128, 2 * SLAB_PX], dt.bfloat16, kind="ExternalInput")
    wmain_d = nc.dram_tensor("wmain", [128, 18 * 256], dt.bfloat16, kind="ExternalInput")
    woff_d = nc.dram_tensor("woff", [128, 2 * 9 * 27], dt.bfloat16, kind="ExternalInput")
    bias_d = nc.dram_tensor("bias", [27, 1], dt.float32, kind="ExternalInput")
    out_d = nc.dram_tensor("out", [NPX, C], dt.float32, kind="ExternalOutput")
    if DEBUG:
        dbg_wicm = nc.dram_tensor("dbg_wicm", [27, NPX], dt.float32, kind="ExternalOutput")
        dbg_idxw = nc.dram_tensor("dbg_idxw", [128, 1152], dt.int16, kind="ExternalOutput")
        dbg_wq = nc.dram_tensor("dbg_wq", [128, 576], dt.bfloat16, kind="ExternalOutput")
        dbg_samp = nc.dram_tensor("dbg_samp", [128, 18 * 512], dt.bfloat16, kind="ExternalOutput")

    # --- inline constants ---
    base_np = np.zeros((128, 144), dtype=np.float32)
    r = np.arange(128)
    for t in range(16):
        for kk in range(KK):
            ki, kj = kk // 3, kk % 3
            base_np[:, t * 9 + kk] = (2 * t + r // 64 + ki + 5) * Wp + (r % 64) + kj + 5
    base_d = nc.inline_tensor(base_np, name="base_tab")
    identb_d = nc.inline_tensor(np.eye(128, dtype=BF16), name="ident_bf")
    identf_d = nc.inline_tensor(np.eye(128, dtype=np.float32), name="ident_f32")

    with tile.TileContext(nc) as tc:
        with tc.tile_pool(name="const", bufs=1) as cpool:
            xcm_sb = cpool.tile([128, 2 * SLAB_PX], dt.bfloat16)
            wmain_sb = cpool.tile([128, 18 * 256], dt.bfloat16)
            woff_sb = cpool.tile([128, 2 * 9 * 27], dt.bfloat16)
            bias_sb = cpool.tile([27, 1], dt.float32)
            base_sb = cpool.tile([128, 144], dt.float32)
            identb_sb = cpool.tile([128, 128], dt.bfloat16)
            identf_sb = cpool.tile([128, 128], dt.float32)
            nc.sync.dma_start(woff_sb[:], woff_d[:])
            nc.sync.dma_start(bias_sb[:], bias_d[:])
            nc.scalar.dma_start(base_sb[:], base_d[:])
            nc.scalar.dma_start(identb_sb[:], identb_d[:])
            nc.scalar.dma_start(identf_sb[:], identf_d[:])
            # xcm split: rows 0-15 first so P3(nt0) can start early
            nc.sync.dma_start(
                bass.AP(xcm_sb.tensor, xcm_sb.offset,
                        [list(xcm_sb.ap[0]), [SLAB_PX, 2], [1, 16 * Wp]]),
                bass.AP(xcm_d, 0, [[2 * SLAB_PX, 128], [SLAB_PX, 2], [1, 16 * Wp]]))
            nc.sync.dma_start(
                bass.AP(xcm_sb.tensor, xcm_sb.offset + 16 * Wp,
                        [list(xcm_sb.ap[0]), [SLAB_PX, 2], [1, SLAB_PX - 16 * Wp]]),
                bass.AP(xcm_d, 16 * Wp,
                        [[2 * SLAB_PX, 128], [SLAB_PX, 2], [1, SLAB_PX - 16 * Wp]]))
            nc.gpsimd.dma_start(wmain_sb[:], wmain_d[:])

            with tc.tile_pool(name="wi", bufs=1) as wipool:
                wi_cm = wipool.tile([27, NPX], dt.float32)
                wi_pm = wipool.tile([128, 16 * 27], dt.float32)

                with tc.tile_pool(name="p4", bufs=1) as p4:
                    o1c = p4.tile([128, 144], dt.float32)
                    o2c = p4.tile([128, 144], dt.float32)
                    fo1 = p4.tile([128, 144], dt.float32)
                    fo2 = p4.tile([128, 144], dt.float32)
                    dy = p4.tile([128, 144], dt.float32)
                    dx = p4.tile([128, 144], dt.float32)
                    dy1 = p4.tile([128, 144], dt.float32)
                    dx1 = p4.tile([128, 144], dt.float32)
                    msig = p4.tile([128, 144], dt.float32)
                    w00 = p4.tile([128, 144], dt.float32)
                    w01 = p4.tile([128, 144], dt.float32)
                    w10 = p4.tile([128, 144], dt.float32)
                    w11 = p4.tile([128, 144], dt.float32)
                    ti32 = p4.tile([128, 144], dt.int32)
                    tf32 = p4.tile([128, 144], dt.float32)
                    gcmp = p4.tile([128, 144], dt.float32)
                    idxf = p4.tile([128, 144], dt.float32)
                    idx16a = p4.tile([128, 36], dt.int16)
                    idx16b = p4.tile([128, 108], dt.int16)
                    idxqa = p4.tile([16, 288], dt.int16)
                    idxqb = p4.tile([16, 864], dt.int16)
                    idxwa = p4.tile([128, 288], dt.int16)
                    idxwb = p4.tile([128, 864], dt.int16)
                    wq = p4.tile([128, 576], dt.bfloat16)

                    v = nc.vector
                    pw = wq.ap[0][0]
                    pid = identb_sb.ap[0][0]

                    def split_tiles(c0):
                        return (idx16a, idxqa, idxwa) if c0 == 0 else \
                               (idx16b, idxqb, idxwb)

                    def p3_band(nt, psA):
                        ps = psA.tile([27, 512], dt.float32, tag="psA")
                        hh = nt * 8
                        first = True
                        for tap in range(9):
                            ki, kj = tap // 3, tap % 3
                            for ch in range(2):
                                lhsT = woff_sb[:, ch * 243 + tap * 27:
                                               ch * 243 + (tap + 1) * 27]
                                off = ch * SLAB_PX + (hh + ki + 5) * Wp + kj + 5
                                rhs = bass.AP(
                                    xcm_sb.tensor, xcm_sb.offset + off,
                                    [list(xcm_sb.ap[0]), [Wp, 8], [1, 64]])
                                nc.tensor.matmul(
                                    ps[:], lhsT, rhs,
                                    start=first, stop=(tap == 8 and ch == 1))
                                first = False
                        nc.scalar.activation(
                            wi_cm[:, nt * 512:(nt + 1) * 512], ps[:],
                            Act.Identity, bias=bias_sb[:, 0:1], scale=1.0)

                    def pt_band(nt, psB):
                        for t in range(4 * nt, 4 * nt + 4):
                            pst = psB.tile([128, 27], dt.float32, tag="psB")
                            nc.tensor.transpose(
                                pst[:], wi_cm[:, t * 128:(t + 1) * 128],
                                identf_sb[0:27, 0:27])
                            nc.scalar.copy(wi_pm[:, t * 27:(t + 1) * 27], pst[:])

                    def wi_view(ch0, c0, nc_):
                        return bass.AP(wi_pm.tensor, wi_pm.offset + ch0 + 27 * c0,
                                       [list(wi_pm.ap[0]), [27, nc_], [1, 9]])

                    def sl(t_, c0, nc_):
                        return t_[:, 9 * c0:9 * (c0 + nc_)]

                    def p4_idx(c0, nc_):
                        # index part first: clamp, floor, gather record index
                        v.tensor_scalar(sl(o1c, c0, nc_), wi_view(0, c0, nc_),
                                        CLAMP, -CLAMP, Alu.min, Alu.max)
                        v.tensor_scalar(sl(o2c, c0, nc_), wi_view(9, c0, nc_),
                                        CLAMP, -CLAMP, Alu.min, Alu.max)
                        v.tensor_copy(sl(ti32, c0, nc_), sl(o1c, c0, nc_))
                        v.tensor_copy(sl(tf32, c0, nc_), sl(ti32, c0, nc_))
                        v.tensor_tensor(sl(gcmp, c0, nc_), sl(tf32, c0, nc_),
                                        sl(o1c, c0, nc_), Alu.is_gt)
                        v.tensor_sub(sl(fo1, c0, nc_), sl(tf32, c0, nc_),
                                     sl(gcmp, c0, nc_))
                        v.tensor_copy(sl(ti32, c0, nc_), sl(o2c, c0, nc_))
                        v.tensor_copy(sl(tf32, c0, nc_), sl(ti32, c0, nc_))
                        v.tensor_tensor(sl(gcmp, c0, nc_), sl(tf32, c0, nc_),
                                        sl(o2c, c0, nc_), Alu.is_gt)
                        v.tensor_sub(sl(fo2, c0, nc_), sl(tf32, c0, nc_),
                                     sl(gcmp, c0, nc_))
                        v.tensor_scalar_mul(sl(idxf, c0, nc_), sl(fo1, c0, nc_), float(Wp))
                        v.tensor_add(sl(idxf, c0, nc_), sl(idxf, c0, nc_),
                                     sl(fo2, c0, nc_))
                        v.tensor_add(sl(idxf, c0, nc_), sl(idxf, c0, nc_),
                                     bass.AP(base_sb.tensor, base_sb.offset + 9 * c0,
                                             [list(base_sb.ap[0]), [1, 9 * nc_]]))
                        # int16 cast in call-major order: col = g_rel*36+kk*4+tl
                        ng = nc_ // 4
                        idx16, _, _ = split_tiles(c0)
                        v.tensor_copy(
                            bass.AP(idx16.tensor, idx16.offset,
                                    [[idx16.ap[0][0], 128], [36, ng], [1, 4], [4, 9]]),
                            bass.AP(idxf.tensor, idxf.offset + 9 * c0,
                                    [[idxf.ap[0][0], 128], [36, ng], [9, 4], [1, 9]]))

                    def p4_w(c0, nc_):
                        # bilinear corner weights, mask-folded
                        nc.scalar.activation(sl(msig, c0, nc_),
                                             wi_view(18, c0, nc_), Act.Sigmoid)
                        v.tensor_sub(sl(dy, c0, nc_), sl(o1c, c0, nc_),
                                     sl(fo1, c0, nc_))
                        v.tensor_sub(sl(dx, c0, nc_), sl(o2c, c0, nc_),
                                     sl(fo2, c0, nc_))
                        v.tensor_scalar(sl(dy1, c0, nc_), sl(dy, c0, nc_),
                                        -1.0, 1.0, Alu.mult, Alu.add)
                        v.tensor_scalar(sl(dx1, c0, nc_), sl(dx, c0, nc_),
                                        -1.0, 1.0, Alu.mult, Alu.add)
                        v.tensor_mul(sl(w00, c0, nc_), sl(dy1, c0, nc_), sl(dx1, c0, nc_))
                        v.tensor_mul(sl(w01, c0, nc_), sl(dy1, c0, nc_), sl(dx, c0, nc_))
                        v.tensor_mul(sl(w10, c0, nc_), sl(dy, c0, nc_), sl(dx1, c0, nc_))
                        v.tensor_mul(sl(w11, c0, nc_), sl(dy, c0, nc_), sl(dx, c0, nc_))
                        v.tensor_mul(sl(w00, c0, nc_), sl(w00, c0, nc_), sl(msig, c0, nc_))
                        v.tensor_mul(sl(w01, c0, nc_), sl(w01, c0, nc_), sl(msig, c0, nc_))
                        v.tensor_mul(sl(w10, c0, nc_), sl(w10, c0, nc_), sl(msig, c0, nc_))
                        v.tensor_mul(sl(w11, c0, nc_), sl(w11, c0, nc_), sl(msig, c0, nc_))

                    def wq_split(c0, nc_):
                        ng = nc_ // 4
                        for cr, wt in enumerate((w00, w10, w01, w11)):
                            v.tensor_copy(
                                bass.AP(wq.tensor, wq.offset + 36 * c0 + cr,
                                        [[pw, 128], [144, ng], [16, 9], [4, 4]]),
                                bass.AP(wt.tensor, wt.offset + 9 * c0,
                                        [[wt.ap[0][0], 128], [36, ng], [1, 9], [9, 4]]))

                    def idx_fold(c0, nc_, engs):
                        # fold 128 partitions -> 16 (contiguous runs)
                        idx16, idxq, _ = split_tiles(c0)
                        w9 = 9 * nc_
                        piq = idxq.ap[0][0]
                        pi16 = idx16.ap[0][0]
                        for q in range(8):
                            engs[q % len(engs)].dma_start(
                                bass.AP(idxq.tensor, idxq.offset + q * w9,
                                        [[piq, 16], [1, w9]]),
                                bass.AP(idx16.tensor,
                                        idx16.offset + 16 * q * pi16,
                                        [[pi16, 16], [1, w9]]))

                    def idx_shuffle(c0, nc_):
                        # idxw[e, g*288+(kk*4+tl)*8+q] = idxq[e, q*9*nc_ + g*36 + kk*4+tl]
                        _, idxq, idxw = split_tiles(c0)
                        ng = nc_ // 4
                        w9 = 9 * nc_
                        v.tensor_copy(
                            bass.AP(idxw.tensor, idxw.offset,
                                    [[idxw.ap[0][0], 16], [288, ng], [8, 36], [1, 8]]),
                            bass.AP(idxq.tensor, idxq.offset,
                                    [[idxq.ap[0][0], 16], [36, ng], [1, 36], [w9, 8]]))

                    def idx_ladder(c0, nc_, engs):
                        _, _, idxw = split_tiles(c0)
                        for j in range(1, 8):
                            engs[(j - 1) % len(engs)].dma_start(
                                idxw[16 * j:16 * (j + 1), :],
                                idxw[0:16, :])

                    def dg_build(g, kk, dgpool):
                        blk = g * 9 + kk
                        dg = dgpool.tile([128, 2048], dt.bfloat16, tag="DG",
                                         name=f"dg_{g}_{kk}")
                        v.tensor_tensor(
                            bass.AP(dg.tensor, dg.offset,
                                    [[dg.ap[0][0], 128], [128, 16], [1, 128]]),
                            bass.AP(identb_sb.tensor, identb_sb.offset,
                                    [[pid, 128], [0, 16], [1, 128]]),
                            bass.AP(wq.tensor, wq.offset + blk * 16,
                                    [[pw, 128], [1, 16], [0, 128]]),
                            Alu.mult)
                        return dg

                    # ---- front-end, split-pipelined ----
                    dgt = {}
                    with tc.tile_pool(name="DG", bufs=12) as dgpool:
                        with tc.tile_pool(name="psA", bufs=2, space="PSUM") as psA, \
                             tc.tile_pool(name="psB", bufs=3, space="PSUM") as psB:
                            p3_band(0, psA)
                            pt_band(0, psB)
                            for nt in (1, 2, 3):
                                p3_band(nt, psA)
                                pt_band(nt, psB)
                            # indices for BOTH splits first, at high priority
                            # so the scheduler never sinks them behind the
                            # diag builds, and every idx DMA traverses the
                            # fabric before gathers saturate it
                            with tc.high_priority():
                                p4_idx(0, 4)
                                idx_fold(0, 4, [nc.sync, nc.scalar])
                                p4_idx(4, 12)
                                idx_shuffle(0, 4)
                                idx_ladder(0, 4, [nc.sync, nc.scalar])
                                idx_fold(4, 12, [nc.sync])
                                idx_shuffle(4, 12)
                                idx_ladder(4, 12, [nc.sync])
                            p4_w(0, 4)
                            wq_split(0, 4)
                            for kk in range(3):
                                dgt[(0, kk)] = dg_build(0, kk, dgpool)
                            p4_w(4, 12)
                            wq_split(4, 12)
                            for kk in range(3, 9):
                                dgt[(0, kk)] = dg_build(0, kk, dgpool)

                        if DEBUG:
                            nc.sync.dma_start(dbg_wicm[:], wi_cm[:])
                            nc.sync.dma_start(dbg_idxw[:, 0:288], idxwa[:])
                            nc.sync.dma_start(dbg_idxw[:, 288:1152], idxwb[:])
                            nc.sync.dma_start(dbg_wq[:], wq[:])

                        # ---------------- P5/P6/P7 main loop ----------------
                        gather_src = bass.AP(xrp_d, 0, [[512, NREC], [1, 1024]])
                        with tc.tile_pool(name="G", bufs=7) as gpool, \
                             tc.tile_pool(name="samp", bufs=8) as spool, \
                             tc.tile_pool(name="osb", bufs=4) as opool, \
                             tc.tile_pool(name="psC", bufs=4, space="PSUM") as psC, \
                             tc.tile_pool(name="psD", bufs=4, space="PSUM") as psD:
                            for g in range(4):
                                po = [psD.tile([128, 256], dt.float32, tag="psD",
                                               name=f"po_{g}_{tl}")
                                      for tl in range(4)]
                                for kk in range(KK):
                                    blk = g * 9 + kk
                                    dg = dgt.pop((g, kk), None)
                                    if dg is None:
                                        dg = dg_build(g, kk, dgpool)
                                    gt = gpool.tile([128, 4, 1024], dt.bfloat16, tag="G")
                                    idxw_g = idxwa if g == 0 else idxwb
                                    lblk = blk if g == 0 else blk - 9
                                    nc.gpsimd.dma_gather(
                                        out_ap=gt[:],
                                        in_ap=gather_src,
                                        idxs_ap=idxw_g[:, lblk * 32:lblk * 32 + 32],
                                        num_idxs=512,
                                        num_idxs_reg=512,
                                        elem_size=1024,
                                        elem_step=512,
                                        queue_num=blk % 4,
                                    )
                                    for ch in range(2):
                                        ps = psC.tile([128, 512], dt.float32, tag="psC")
                                        for tl in range(4):
                                            for cr in range(4):
                                                nc.tensor.matmul(
                                                    ps[:, tl * 128:(tl + 1) * 128],
                                                    gt[:, tl, cr * 256 + ch * 128:
                                                       cr * 256 + ch * 128 + 128],
                                                    dg[:, (tl * 4 + cr) * 128:
                                                       (tl * 4 + cr + 1) * 128],
                                                    start=(cr == 0), stop=(cr == 3))
                                        st = spool.tile([128, 512], dt.bfloat16,
                                                        tag="samp")
                                        nc.scalar.copy(st[:], ps[:])
                                        if DEBUG and g == 0:
                                            bs = (kk * 2 + ch) * 512
                                            nc.sync.dma_start(
                                                dbg_samp[:, bs:bs + 512], st[:])
                                        # incremental stage-2
                                        n = kk * 2 + ch
                                        for tl in range(4):
                                            nc.tensor.matmul(
                                                po[tl][:],
                                                st[:, tl * 128:(tl + 1) * 128],
                                                wmain_sb[:, n * 256:(n + 1) * 256],
                                                start=(n == 0), stop=(n == 17))
                                for tl in range(4):
                                    ot = opool.tile([128, 256], dt.float32, tag="osb")
                                    nc.scalar.copy(ot[:], po[tl][:])
                                    row0 = (g * 4 + tl) * 128
                                    nc.sync.dma_start(out_d[row0:row0 + 128, :], ot[:])
    nc.finalize()
    return nc


def _host_prep(x, w_offset, b_offset, filt):
    xp = np.zeros((B, 77, Wp, C), dtype=BF16)
    xp[:, PAD:PAD + H, PAD:PAD + W, :] = x.astype(BF16)

    Wm = np.ascontiguousarray(filt.reshape(F, C, KK))
    wmain = np.zeros((128, 18 * 256), dtype=BF16)
    for kk in range(KK):
        for ch in range(2):
            g = kk * 2 + ch
            wmain[:, g * 256:(g + 1) * 256] = Wm[:, ch * 128:(ch + 1) * 128, kk].T.astype(BF16)

    woff = np.zeros((128, 2 * 9 * 27), dtype=BF16)
    for ch in range(2):
        for tap in range(9):
            ki, kj = tap // 3, tap % 3
            woff[:, ch * 243 + tap * 27:ch * 243 + (tap + 1) * 27] = \
                w_offset[ki, kj, ch * 128:(ch + 1) * 128, :].astype(BF16)

    bias = np.ascontiguousarray(b_offset.reshape(27, 1).astype(np.float32))

    in_maps = []
    for core in range(8):
        b, half = core // 2, core % 2
        h0 = 32 * half
        slab = np.ascontiguousarray(xp[b, h0:h0 + SLAB_ROWS].reshape(SLAB_PX, C))
        # paired-row records: rec r = [slab[r], slab[r+76]]
        xrp = np.zeros((NREC + 1, 512), dtype=BF16)
        xrp[:NREC, 0:256] = slab[:NREC]
        xrp[:NREC, 256:512] = slab[Wp:NREC + Wp]
        cm = np.empty((128, 2 * SLAB_PX), dtype=BF16)
        cm[:, 0:SLAB_PX] = slab[:, 0:128].T
        cm[:, SLAB_PX:] = slab[:, 128:256].T
        in_maps.append({
            "xrp": xrp,
            "xslab_cm": np.ascontiguousarray(cm),
            "wmain": wmain,
            "woff": woff,
            "bias": bias,
        })
    return in_maps


def kernel(x, w_offset, b_offset, filt):
    global LAST_RESULT
    x = np.asarray(x, dtype=np.float32)
    w_offset = np.asarray(w_offset, dtype=np.float32)
    b_offset = np.asarray(b_offset, dtype=np.float32)
    filt = np.asarray(filt, dtype=np.float32)

    if "nc" not in _CACHE:
        _CACHE["nc"] = _build_nc()
    nc = _CACHE["nc"]

    from concourse.bass_utils import run_bass_kernel_spmd

    in_maps = _host_prep(x, w_offset, b_offset, filt)
    res = run_bass_kernel_spmd(nc, in_maps, core_ids=list(range(8)))
    LAST_RESULT = res

    out = np.zeros((B, H, W, F), dtype=np.float32)
    for core in range(8):
        b, half = core // 2, core % 2
        out[b, 32 * half:32 * half + 32] = res.results[core]["out"].reshape(32, 64, F)
    return out
